# revision 36
# baseline (speedup 1.0000x reference)
"""Trainium2 Bass kernel: attention with rotary embedding + XL memory (v3.3).

Model (B=2, T=1024, D=2048, H=16, hd=128, XL=1024):
  qkv = x @ w_qkv.T ; split q,k,v ; k_xl += pos_emb ; rope(q), rope(k)
  per head: scores = q @ [k_xl | k].T / sqrt(hd) ; softmax ; y = P @ [v_xl | v]
  out = y @ w_proj.T
sharding: 8 cores = 2 batches x 4 head-groups; host sums the 4 partial
output projections per batch.

v3 vs the 178.8us v2 (fp16-everywhere) kernel: the cost model charges
fp8e4/e5 DoubleRow matmuls 0.5 cycles per output row while packing TWO
128-deep k-tiles per instruction -- 4x fp16 MAC throughput. Straight fp8 is
numerically unusable (e4m3 ~2.7% RMS/element busts the 2e-2 gate), so the
big GEMMs use a residual-compensated decomposition:
    A@B ~= A8@B8 + Ar8@B8 + A8@Br8   (Ar8 = fp8(A - A8), cross term dropped)
Three DoubleRow instructions per two k-tiles = 0.75x fp16 cycles with
~1e-3 accuracy (device-validated). Applied to the qkv q/k projection, the
v projection, and the output projection (contractions 2048/2048/512). The
scores and AV matmuls keep fp16: their single 128-deep k-tile would make
DoubleRow pay parity-or-worse, and quantizing exp outputs on-chip is
engine-prohibitive.
  - all fp8 operand pairs except y are quantized on the HOST (x, w_qkv, wv,
    w_proj); main+residual are stacked along an `mr` axis of ONE dram
    tensor so each prefetch DMA covers both (fp8 pair == fp16 bytes).
  - scale management: weights are pre-scaled x64 on host so both fp8 tensors
    and their unscaled residuals sit in e4m3's normal range; the 1/64 is
    folded into the rope cos/sin tables (q/k), a DVE copy scale (v), and
    the host-side unshard divide (proj output is stored as 2048*out). The
    softmax denominator's `ones` reduction vector is 1/32 so the
    reciprocal broadcast yields 32/den and y16 = py*rbc = 32*y ~ unit RMS,
    putting y8's residual in fp8 range.
  - y is the only on-chip quantization: per (head, tb), y16 = py*rbc (DVE),
    y8 = cast (Pool mid-attention / ACT at the tail), yr8 = y16-y8 (DVE).

v3.3 schedule notes (all cost-model-trace driven; 178.8 -> 168.2us):
  - Pool/SWDGE descriptor generation is ~1.1us per dma_start and serial on
    the Pool engine, so prefetches are COALESCED (one DMA per wqk f-group
    covering main+resid, one for wv/wproj/vxl). w-f2 rides the fast
    SP/HWDGE queue head ahead of the x tb0 pieces; x tb1 + kxl sit BEHIND
    the weight stream on Pool so their transfers cannot steal bus from
    w f3..f7 (measured 4.9us PE stall when they issued early from SP).
  - phase-1 PE order tb0 f2..f7 -> tb0 f0/f1 (2-chain) -> tb1 f4/f5/f6
    (3-chain) -> tb1 f7,f0..f3: single chains consume only the weight
    stream (~200GB/s) while x tb0 lands; the x-hungry interleaved chains
    run once their tb's x is resident; k-groups of tb1 precede q-groups so
    attention tb0 (which reads k of both tbs) never waits on q-tb1 ropes.
  - all rope c16 PSUM->fp16 casts run on ACT: a DVE c16 at the phase tail
    holds PSUM slots hostage behind the DVE backlog and stalls the first
    attention scores (measured 2.7us).
  - per-head attention tail is fused (den+reciprocal -> deferred AVs ->
    broadcast -> normalize -> y8 cast -> yr8 sub) so head h's Pool/DVE/ACT
    chain drains under head h+1's AV matmuls; reserve=8 proj-tb0 fillers
    run after the chains to cover the ACT-backlog-gated tail before the
    proj-tb1 blocks need y8/yr8 (was a 4.3us PE stall at reserve=4).
  - ALL v-gemm groups fill attention-tb0 chunk slots (PE 2984ns vs ACT 4
    exps 2448ns per slot); proj-tb0 blocks fill attention tb1 starting at
    slot 3 (earlier ones would block the in-order PE queue on the tb0
    y-quant chain); out-DMAs alternate SP/Pool queues at the tail.
  Measured dead ends kept out: stream-interleaved SP byte order and a
  2-chain f2/f3 front (+12us), reserved fillers before the denominators
  (delays the AV->quant critical path, +4.6us), paired out-DMAs with an
  interleaved out layout (non-coalescable partition rows, +4us), outp pool
  below 8 bufs (DMA-completion rotation throttles proj blocks, +4us),
  denominators packed into one PSUM bank at partition offsets 0/32/64/96
  (walrus codegen rejects offset matmul outputs).
"""
import sys

sys.path.insert(0, "/opt/trn_rl_repo")

import numpy as np
import ml_dtypes

import concourse.bass as bass  # noqa: F401
import concourse.mybir as mybir
import concourse.tile as tile
from concourse import bacc
from concourse.bass import ts
from concourse.bass_utils import run_bass_kernel_spmd  # noqa: F401 (fallback)

F32 = mybir.dt.float32
F16 = mybir.dt.float16
F8 = mybir.dt.float8e4
AF = mybir.ActivationFunctionType
DR = mybir.MatmulPerfMode.DoubleRow
SUB = mybir.AluOpType.subtract
E4 = ml_dtypes.float8_e4m3

B, T, D = 2, 1024, 2048
H, HD, XL = 16, 128, 1024
HPC = 4                 # heads per core
CPB = 4                 # cores per batch
NCORES = 8
NCC = D // 128          # 16 contraction chunks (8 DoubleRow pairs)
NCJ = NCC // 2
SCALE = 1.0 / np.sqrt(HD)
WS = 64.0               # host weight prescale (folded back downstream)
YS = 32.0               # y prescale via ones=1/YS denominator reduction

_CACHE: dict = {}


def _build_nc():
    nc = bacc.Bacc("TRN2", target_bir_lowering=False, debug=False)

    x_d = nc.dram_tensor("x", [2, 128, 2, NCJ, 2, 512], F8,
                         kind="ExternalInput")
    wqk_d = nc.dram_tensor("wqk", [8, 128, 2, NCJ, 2, 128], F8,
                           kind="ExternalInput")
    wv_d = nc.dram_tensor("wv", [128, 2, NCJ, 2, 512], F8,
                          kind="ExternalInput")
    cs_d = nc.dram_tensor("cs", [2, 128, T], F16, kind="ExternalInput")
    kxl_d = nc.dram_tensor("kxl", [128, 4, XL], F16, kind="ExternalInput")
    vxl_d = nc.dram_tensor("vxl", [128, 8, 512], F16, kind="ExternalInput")
    wp_d = nc.dram_tensor("wp", [128, 2, 16, 2, 2, 128], F8,
                          kind="ExternalInput")
    out_d = nc.dram_tensor("out", [16, 2, 128, 512], F16, kind="ExternalOutput")

    gp = nc.gpsimd
    with tile.TileContext(nc) as tc, nc.allow_low_precision(
            reason="fp8 DoubleRow residual-compensated pipeline: ~2e-3 rel "
                   "err, gate is 2e-2"):
        with (
            tc.tile_pool(name="const", bufs=1) as const,
            tc.tile_pool(name="ropep", bufs=3) as ropep,
            tc.tile_pool(name="ptp", bufs=17) as ptp,
            tc.tile_pool(name="accp", bufs=8) as accp,
            tc.tile_pool(name="smallp", bufs=6) as smallp,
            tc.tile_pool(name="rbcp", bufs=4) as rbcp,
            tc.tile_pool(name="ynp", bufs=4) as ynp,
            tc.tile_pool(name="outp", bufs=8) as outp,
            tc.tile_pool(name="psum", bufs=4, space="PSUM") as psum,
            tc.tile_pool(name="pyp", bufs=4, space="PSUM") as pyp,
        ):
            # ---- persistent tiles (everything resident once loaded) ----
            cc = const.tile([128, T], F16, tag="cc")    # [cos; cos] / 64
            ss = const.tile([128, T], F16, tag="ss")    # [-sin; +sin] / 64
            ones = const.tile([128, 128], F16, tag="ones")  # = 1/YS
            qk = const.tile([128, 8, T], F16, tag="qk")   # roped qT 0-3, kT 4-7
            vsb = const.tile([128, 8, 512], F16, tag="vsb")  # v [t, d] natural
            y8sb = const.tile([128, 4, T], F8, tag="y8")     # 32*y fp8 main
            yr8sb = const.tile([128, 4, T], F8, tag="yr8")   # 32*y fp8 resid
            xB = const.tile([128, 2, 2, NCJ, 2, 512], F8, tag="x")
            wqkB = const.tile([128, 8, 2, NCJ, 2, 128], F8, tag="wqk")
            wvB = const.tile([128, 2, NCJ, 2, 512], F8, tag="wv")
            kxl = const.tile([128, 4, XL], F16, tag="kxl")
            vxl = const.tile([128, 8, 512], F16, tag="vxl")
            wpB = const.tile([128, 2, 16, 2, 2, 128], F8, tag="wp")

            gp.memset(ones[:], 1.0 / YS)

            # PE p-state warmup: dummy matmuls on `ones` while the first
            # DMAs land, so real matmuls start at the full 2.4GHz p-state
            # (the ramp needs ~3us of continuous PE busy). The dummy exp
            # pulls the ACT function-table load (1.3us) off the first real
            # exp at attention start.
            warm16 = ropep.tile([128, 512], F16, tag="c16", name="w16")
            for wu in range(3):
                pw = psum.tile([128, 512], F32, tag="ps", name="warm")
                for _ in range(13):
                    nc.tensor.matmul(pw[:, 0:128], ones[:], ones[:],
                                     start=True, stop=True)
                if wu == 0:
                    nc.scalar.activation(warm16[0:1, 0:128], pw[0:1, 0:128],
                                         AF.Exp, scale=SCALE)

            # ---- prefetch, priority order matched to the phase-1 PE order.
            # SP/HWDGE (565ns issue): x tb0 stream, then x tb1 + kxl.
            # Pool/SWDGE (~1.1us gen per DMA, serial on Pool): weights in
            # f-need order; each DMA covers a main+resid pair.
            # Stream-aligned prefetch: SP/HWDGE (565ns issue) carries the
            # phase-1-critical bytes in exact CONSUMPTION order (w f2, x
            # pieces interleaved with w f3..f7, then x tb1 + kxl), so the
            # front 2-chain is never byte-starved; Pool/SWDGE (~1.1us gen
            # per DMA, serial on Pool) trickles the rest.
            nc.sync.dma_start(wqkB[:, 2], wqk_d[2])
            for p2 in range(4):                      # x tb0, 2-j pieces
                nc.sync.dma_start(xB[:, 0, :, 2 * p2:2 * p2 + 2],
                                  x_d[0, :, :, 2 * p2:2 * p2 + 2])
            for f in (3, 4, 5, 6, 7):
                gp.dma_start(wqkB[:, f], wqk_d[f])
            gp.dma_start(cc[:], cs_d[0])
            gp.dma_start(ss[:], cs_d[1])
            gp.dma_start(wqkB[:, 0], wqk_d[0])
            gp.dma_start(wqkB[:, 1], wqk_d[1])
            gp.dma_start(xB[:, 1], x_d[1])           # x tb1 whole
            gp.dma_start(kxl[:], kxl_d[:])
            gp.dma_start(wvB[:], wv_d[:])
            gp.dma_start(vxl[:], vxl_d[:])
            gp.dma_start(wpB[:], wp_d[:])

            # ---- phase 1: q/k projection + rope ----
            def qk_mms_j(pmm, f, tb, j):
                # residual-compensated fp8 DoubleRow: one 256-deep k-tile
                # pair per instruction, 3 instructions per pair
                nc.tensor.matmul(pmm[:], wqkB[:, f, 0, j], xB[:, tb, 0, j],
                                 start=(j == 0), stop=False, perf_mode=DR)
                nc.tensor.matmul(pmm[:], wqkB[:, f, 1, j], xB[:, tb, 0, j],
                                 start=False, stop=False, perf_mode=DR)
                nc.tensor.matmul(pmm[:], wqkB[:, f, 0, j], xB[:, tb, 1, j],
                                 start=False, stop=(j == NCJ - 1),
                                 perf_mode=DR)

            def emit_rope(pmm, f, tb):
                # packed rope: new = P*[cos;cos] + swap(P)*[-sin;+sin].
                # cc/ss carry the 1/WS weight-prescale compensation, so
                # c16 holds 64*q and dst comes out at natural scale. c16
                # (ACT) casts PSUM->fp16 so the DVE combine runs in 4x
                # packed mode; the half-swap copies run on Pool.
                tbsl = ts(tb, 512)
                c16 = ropep.tile([128, 512], F16, tag="c16")
                nc.scalar.copy(c16[:], pmm[:])
                sw = ropep.tile([128, 512], F16, tag="sw")
                gp.tensor_copy(sw[0:64, :], c16[64:128, :])
                gp.tensor_copy(sw[64:128, :], c16[0:64, :])
                dst = qk[:, f, tbsl]
                t2 = ropep.tile([128, 512], F16, tag="t2")
                nc.vector.tensor_mul(dst, c16[:], cc[:, tbsl])
                nc.vector.tensor_mul(t2[:], sw[:], ss[:, tbsl])
                nc.vector.tensor_add(dst, dst, t2[:])

            def qk_group_chains(specs):
                # interleaved f-chains: PE consumes the incoming x/w byte
                # stream no faster than the 360GB/s transfer unit delivers
                pms = {}
                for fx, tbx in specs:
                    pms[(fx, tbx)] = psum.tile([128, 512], F32, tag="ps",
                                               name=f"pm{fx}{tbx}")
                for j in range(NCJ):
                    for fx, tbx in specs:
                        qk_mms_j(pms[(fx, tbx)], fx, tbx, j)
                for fx, tbx in specs:
                    emit_rope(pms[(fx, tbx)], fx, tbx)

            # tb0: f2/f3 as a 2-chain paced to the arriving x tb0 stream
            # (a single chain consumes x at 780GB/s vs the ~300GB/s bus),
            # f4..f7 single once x is resident, then f0/f1. tb1: f4/f5/f6
            # as a 3-chain paced to the x tb1 stream, k-groups (f4..f7)
            # before q-groups so attention tb0 (which needs k of both tbs)
            # never waits on q-tb1 ropes.
            qk_group_chains([(2, 0), (3, 0)])
            for f in range(4, 8):
                qk_group_chains([(f, 0)])
            qk_group_chains([(0, 0), (1, 0)])
            qk_group_chains([(4, 1), (5, 1), (6, 1)])
            for f in (7, 0, 1, 2, 3):
                qk_group_chains([(f, 1)])

            # v in natural [t, d] layout. ALL v-gemm groups are deferred into
            # the attention-tb0 chunk slots (PE gap fillers). Half-width
            # (256 v-cols) gives 16 fillers for 16 slots; column half `hf`
            # covers heads 2hf..2hf+1. The PSUM->SBUF copy runs on DVE
            # (tensor_scalar 1/WS) because ACT's exp headroom in tb0 slots
            # is thin with the 0.75x fp8 fillers.
            def emit_v_group(tb, tt, hf):
                pv = psum.tile([128, 256], F32, tag="ps", name="pv")
                for j in range(NCJ):
                    nc.tensor.matmul(pv[:], xB[:, tb, 0, j, :, ts(tt, 128)],
                                     wvB[:, 0, j, :, ts(hf, 256)],
                                     start=(j == 0), stop=False, perf_mode=DR)
                    nc.tensor.matmul(pv[:], xB[:, tb, 1, j, :, ts(tt, 128)],
                                     wvB[:, 0, j, :, ts(hf, 256)],
                                     start=False, stop=False, perf_mode=DR)
                    nc.tensor.matmul(pv[:], xB[:, tb, 0, j, :, ts(tt, 128)],
                                     wvB[:, 1, j, :, ts(hf, 256)],
                                     start=False, stop=(j == NCJ - 1),
                                     perf_mode=DR)
                nc.vector.tensor_scalar_mul(vsb[:, tb * 4 + tt, ts(hf, 256)],
                                            pv[:], 1.0 / WS)

            v_fillers = [
                lambda tb=tb, tt=tt, hf=hf: emit_v_group(tb, tt, hf)
                for tb in range(2) for tt in range(4) for hf in range(2)]

            # ---- phase 2: attention + projection, interleaved ----
            def emit_proj(ob, tb, on_act=False, dma_gp=False):
                tbsl = ts(tb, 512)
                po = psum.tile([128, 512], F32, tag="ps")
                for yj in range(2):
                    y8p = y8sb[:, 2 * yj:2 * yj + 2, tbsl]
                    yr8p = yr8sb[:, 2 * yj:2 * yj + 2, tbsl]
                    nc.tensor.matmul(po[:], wpB[:, 0, ob, yj], y8p,
                                     start=(yj == 0), stop=False, perf_mode=DR)
                    nc.tensor.matmul(po[:], wpB[:, 1, ob, yj], y8p,
                                     start=False, stop=False, perf_mode=DR)
                    nc.tensor.matmul(po[:], wpB[:, 0, ob, yj], yr8p,
                                     start=False, stop=(yj == 1), perf_mode=DR)
                ot = outp.tile([128, 512], F16, tag="ot")
                # out is stored as WS*YS*out = 2048*out; host divides.
                # Pool/GPSIMD cannot read PSUM: fillers copy on DVE (ACT is
                # exp-saturated mid-attention); the tail copies on ACT.
                if on_act:
                    nc.scalar.copy(ot[:], po[:])
                else:
                    nc.vector.tensor_copy(ot[:], po[:])
                # the 16-block tail would queue 9us of serial SP DMA issue;
                # alternate the idle Pool/SWDGE queue to halve it
                if dma_gp:
                    gp.dma_start(out_d[ob, tb], ot[:])
                else:
                    nc.sync.dma_start(out_d[ob, tb], ot[:])

            def attn_quad(tb, fillers, every=2, reserve=0, act_quant=False):
                """Chunk-interleaved attention for all 4 heads; `fillers`
                are callables (or None placeholders) emitted inside chunk
                slots (PE gap fillers). The per-head tail (denominator,
                deferred AVs, reciprocal broadcast, normalize, y8/yr8 fp8
                quantization) is fused per head so head h's norm/quant chain
                (Pool/DVE/ACT) drains under head h+1..3's AV matmuls; the
                last `reserve` fillers run after it to cover the tail."""
                tbsl = ts(tb, 512)
                py, acc = {}, {}
                for h in range(4):
                    py[h] = pyp.tile([128, 512], F32, tag="py", name=f"py{h}")
                    acc[h] = accp.tile([128, 512], F16, tag="acc",
                                       name=f"acc{h}")
                fill = list(fillers)
                pend = {h: [] for h in range(4)}   # av deferred 3 chunks
                def emit_av(h):
                    pt_, lv_, kc_ = pend[h].pop(0)
                    nc.tensor.matmul(py[h][:], lv_, pt_[:],
                                     start=(kc_ == 0), stop=(kc_ == 15))
                for kc in range(16):
                    for h in range(4):
                        if kc < 8:
                            lk = kxl[:, h, ts(kc, 128)]
                            lv = vxl[:, kc, ts(h, 128)]
                        else:
                            lk = qk[:, 4 + h, ts(kc - 8, 128)]
                            lv = vsb[:, kc - 8, ts(h, 128)]
                        pss = psum.tile([128, 512], F32, tag="ps")
                        nc.tensor.matmul(pss[:], lk, qk[:, h, tbsl],
                                         start=True, stop=True)
                        pt = ptp.tile([128, 512], F16, tag="pt")
                        nc.scalar.activation(pt[:], pss[:], AF.Exp, scale=SCALE)
                        if kc == 0:
                            nc.vector.tensor_copy(acc[h][:], pt[:])
                        else:
                            nc.vector.tensor_add(acc[h][:], acc[h][:], pt[:])
                        if len(pend[h]) >= 3:
                            emit_av(h)
                        pend[h].append((pt, lv, kc))
                    if kc % every == every - 1 and len(fill) > reserve:
                        f = fill.pop(0)
                        if f is not None:
                            f()
                # denominators first (acc is final after kc15's add) so the
                # DVE reciprocals hide under the final AV matmuls. ones=1/YS
                # makes rec = YS/den so y16 = py*rbc = YS*y ~ unit RMS.
                recs = []
                for h in range(4):
                    pden_t = psum.tile([128, 512], F32, tag="ps")
                    nc.tensor.matmul(pden_t[0:1, :], ones[:, 0:1], acc[h][:],
                                     start=True, stop=True)
                    rec = smallp.tile([1, 512], F16, tag="rec")
                    nc.vector.reciprocal(rec[:], pden_t[0:1, :])
                    recs.append(rec)
                # per-head tail: deferred AVs -> reciprocal broadcast on Pool
                # -> normalize (frees the py bank) -> y8 cast (Pool mid-
                # attention, ACT at the tail) -> yr8 residual on DVE. Head
                # h's non-PE chain hides under head h+1's AV matmuls.
                for h in range(4):
                    while pend[h]:
                        emit_av(h)
                    rbc = rbcp.tile([128, 512], F16, tag="rbc")
                    gp.partition_broadcast(rbc[:], recs[h][:])
                    y16 = ynp.tile([128, 512], F16, tag="y16")
                    nc.vector.tensor_mul(y16[:], py[h][:], rbc[:])
                    dst8 = y8sb[:, h, tbsl]
                    if act_quant:
                        nc.scalar.copy(dst8, y16[:])
                    else:
                        gp.tensor_copy(dst8, y16[:])
                    nc.vector.tensor_tensor(yr8sb[:, h, tbsl], y16[:],
                                            dst8, SUB)
                # reserved fillers: independent PE work emitted after the
                # per-head chains to cover their Pool/DVE/ACT latency
                while fill:
                    f = fill.pop(0)
                    if f is not None:
                        f()

            attn_quad(0, v_fillers, every=1)
            # proj-tb0 fillers skip the first 3 tb1 slots (tb0's y8/yr8
            # quant chain is still draining; an early proj matmul would
            # block the in-order PE queue on it)
            # reserve=8: the tb1 y-quant chain is gated by the ACT exp
            # backlog (~5us); 8 reserved proj-tb0 blocks of PE work cover it
            attn_quad(1, [None] * 3
                      + [lambda ob=ob: emit_proj(ob, 0) for ob in range(16)],
                      every=1, reserve=8, act_quant=True)
            # proj tb1 (tail): groups of 4 obs, all yj0 (heads 0/1) triplets
            # of a group before any yj1, so the in-order PE makes progress
            # while h2/h3's y-quant chain is still draining
            tb1sl = ts(1, 512)
            for g in range(4):
                obs = list(range(4 * g, 4 * g + 4))
                pos = {}
                for yj in range(2):
                    y8p = y8sb[:, 2 * yj:2 * yj + 2, tb1sl]
                    yr8p = yr8sb[:, 2 * yj:2 * yj + 2, tb1sl]
                    for ob in obs:
                        if yj == 0:
                            pos[ob] = psum.tile([128, 512], F32, tag="ps",
                                                name=f"po{ob}")
                        po = pos[ob]
                        nc.tensor.matmul(po[:], wpB[:, 0, ob, yj], y8p,
                                         start=(yj == 0), stop=False,
                                         perf_mode=DR)
                        nc.tensor.matmul(po[:], wpB[:, 1, ob, yj], y8p,
                                         start=False, stop=False,
                                         perf_mode=DR)
                        nc.tensor.matmul(po[:], wpB[:, 0, ob, yj], yr8p,
                                         start=False, stop=(yj == 1),
                                         perf_mode=DR)
                for ob in obs:
                    ot = outp.tile([128, 512], F16, tag="ot")
                    nc.scalar.copy(ot[:], pos[ob][:])
                    if ob % 2 == 0:
                        gp.dma_start(out_d[ob, 1], ot[:])
                    else:
                        nc.sync.dma_start(out_d[ob, 1], ot[:])

    nc.compile()
    return nc


def _get_nc():
    if "nc" not in _CACHE:
        _CACHE["nc"] = _build_nc()
    return _CACHE["nc"]


_PERM = np.concatenate([np.arange(0, HD, 2), np.arange(1, HD, 2)])
_PP = np.concatenate([_PERM + i * HD for i in range(HPC)])  # per-head-block


def _q8mr(a, axis):
    """fp8 e4m3 main + unscaled residual, stacked along a new `mr` axis."""
    a8 = a.astype(E4)
    ar8 = (a - a8.astype(np.float32)).astype(E4)
    return np.ascontiguousarray(np.stack([a8, ar8], axis=axis))


def make_in_maps(x, cos, sin, k_xl, v_xl, pos_emb, w_qkv, w_proj):
    """Host-side shard + layout prep + fp8 quantization: one input dict per
    core."""
    x = np.asarray(x, np.float32)
    cos = np.asarray(cos, np.float32)
    sin = np.asarray(sin, np.float32)
    k_xl = np.asarray(k_xl, np.float32) + np.asarray(pos_emb, np.float32)
    v_xl = np.asarray(v_xl, np.float32)
    w_qkv = np.asarray(w_qkv, np.float32)
    w_proj = np.asarray(w_proj, np.float32)

    # cs[0] = [cos; cos]/WS ; cs[1] = [-sin; +sin]/WS  (packed-rope factors
    # with the w_qkv x64 prescale folded back)
    cs = np.ascontiguousarray(np.stack([
        np.concatenate([cos.T, cos.T], axis=0),
        np.concatenate([-sin.T, sin.T], axis=0),
    ]) / WS).astype(np.float16)

    in_maps = []
    for c in range(NCORES):
        b, g = c // CPB, c % CPB
        h0 = g * HPC
        cols = slice(h0 * HD, (h0 + HPC) * HD)

        # x: [tb, pi, mr, cj, kt, tl] fp8 pair
        x_arr = np.ascontiguousarray(
            x[b].T.reshape(NCJ, 2, 128, 2, 512).transpose(3, 2, 0, 1, 4))
        x_q = _q8mr(x_arr, 2)
        # w_q/w_k rows for this head group, rope-permuted, x WS;
        # [f, pi, mr, cj, kt, fcol] fp8 pair
        wq = w_qkv[0 * D + h0 * HD:0 * D + (h0 + HPC) * HD][_PP]
        wk = w_qkv[1 * D + h0 * HD:1 * D + (h0 + HPC) * HD][_PP]
        wqk_rows = np.concatenate([wq, wk], axis=0) * WS  # [1024, D]
        wqk_arr = np.ascontiguousarray(
            wqk_rows.reshape(8, 128, NCJ, 2, 128).transpose(0, 4, 2, 3, 1))
        wqk_q = _q8mr(wqk_arr, 2)
        # w_v rows (unpermuted) x WS; [pi, mr, cj, kt, vcol] fp8 pair
        wv_rows = w_qkv[2 * D + h0 * HD:2 * D + (h0 + HPC) * HD] * WS
        wv_arr = np.ascontiguousarray(
            wv_rows.T.reshape(NCJ, 2, 128, 512).transpose(2, 0, 1, 3))
        wv_q = _q8mr(wv_arr, 1)
        # k_xl (pos already added): permuted cols, transposed; [pi, j, t]
        kxlT = k_xl[b][:, cols][:, _PP].T  # [512, XL]
        kxl_arr = np.ascontiguousarray(
            kxlT.reshape(4, 128, XL).transpose(1, 0, 2)).astype(np.float16)
        # v_xl natural; [pi, j, col]
        vxl_arr = np.ascontiguousarray(
            v_xl[b][:, cols].reshape(8, 128, 512).transpose(1, 0, 2)
        ).astype(np.float16)
        # w_proj column block, transposed, x WS; [pi, mr, ob, ycj, kt, ocol]
        wprojT = w_proj[:, cols].T * WS  # [512, D]
        wp_arr = np.ascontiguousarray(
            wprojT.reshape(2, 2, 128, 16, 128).transpose(2, 3, 0, 1, 4))
        wp_q = _q8mr(wp_arr, 1)

        in_maps.append({
            "x": x_q, "wqk": wqk_q, "wv": wv_q, "cs": cs,
            "kxl": kxl_arr, "vxl": vxl_arr, "wp": wp_q,
        })
    return in_maps


def unshard(results):
    """results: list of 8 dicts with 'out' [16, 2, 128, 512] -> [B, T, D].
    Device partials are scaled by WS*YS; divide once after summing."""
    out = np.zeros((B, T, D), np.float32)
    for c in range(NCORES):
        b = c // CPB
        outT = np.asarray(results[c]["out"]).transpose(0, 2, 1, 3).reshape(D, T)
        out[b] += outT.T
    out /= (WS * YS)
    return out


def _get_runner():
    """Persistent jitted 8-core executable (avoids per-call retrace of the
    bass2jax lowering; the NEFF itself is cached by neuronx-cc)."""
    if "runner" in _CACHE:
        return _CACHE["runner"]
    import jax
    import jax.numpy as jnp
    from jax.sharding import Mesh, PartitionSpec, NamedSharding
    from jax.experimental.shard_map import shard_map
    from concourse.bass2jax import (_bass_exec_p, partition_id_tensor,
                                    install_neuronx_cc_hook)

    nc = _get_nc()
    install_neuronx_cc_hook()
    in_names, out_names, out_avals, zero_shapes = [], [], [], []
    for alloc in nc.m.functions[0].allocations:
        if not isinstance(alloc, mybir.MemoryLocationSet):
            continue
        name = alloc.memorylocations[0].name
        if alloc.kind == "ExternalInput":
            if nc.partition_id_tensor is None or \
                    name != nc.partition_id_tensor.name:
                in_names.append(name)
        elif alloc.kind == "ExternalOutput":
            shape = tuple(alloc.tensor_shape)
            np_dt = mybir.dt.np(alloc.dtype)
            out_names.append(name)
            out_avals.append(jax.core.ShapedArray(shape, np_dt))
            zero_shapes.append((shape, np_dt))
    n_params, n_outs = len(in_names), len(out_names)
    all_in = in_names + out_names
    if nc.partition_id_tensor is not None:
        all_in = all_in + [nc.partition_id_tensor.name]

    def _body(*args):
        operands = list(args)
        if nc.partition_id_tensor is not None:
            operands.append(partition_id_tensor())
        return tuple(_bass_exec_p.bind(
            *operands, out_avals=tuple(out_avals), in_names=tuple(all_in),
            out_names=tuple(out_names), lowering_input_output_aliases=(),
            sim_require_finite=True, sim_require_nnan=True, nc=nc))

    devices = jax.devices()[:NCORES]
    mesh = Mesh(np.asarray(devices), ("core",))
    fn = jax.jit(
        shard_map(_body, mesh=mesh,
                  in_specs=(PartitionSpec("core"),) * (n_params + n_outs),
                  out_specs=(PartitionSpec("core"),) * n_outs,
                  check_rep=False),
        donate_argnums=tuple(range(n_params, n_params + n_outs)),
        keep_unused=True)
    sharding = NamedSharding(mesh, PartitionSpec("core"))
    zfn = jax.jit(
        lambda: tuple(jnp.zeros((NCORES * s[0], *s[1:]), d)
                      for s, d in zero_shapes),
        out_shardings=(sharding,) * n_outs)
    runner = (fn, zfn, in_names, out_names, out_avals, sharding)
    _CACHE["runner"] = runner
    return runner


def kernel(x, cos, sin, k_xl, v_xl, pos_emb, w_qkv, w_proj, is_causal=0,
           **_ignored):
    # is_causal is 0 for this problem spec (fill=arange, shape []); the
    # non-causal path is the only one implemented.
    import jax
    in_maps = make_in_maps(x, cos, sin, k_xl, v_xl, pos_emb, w_qkv, w_proj)
    fn, zfn, in_names, out_names, out_avals, sharding = _get_runner()
    concat_in = [
        jax.device_put(
            np.concatenate([in_maps[c][nm] for c in range(NCORES)], axis=0),
            sharding)
        for nm in in_names]
    outs = fn(*concat_in, *zfn())
    results = [
        {nm: np.asarray(outs[i]).reshape(NCORES, *out_avals[i].shape)[c]
         for i, nm in enumerate(out_names)}
        for c in range(NCORES)]
    _CACHE["last_results"] = None
    return unshard(results)


# revision 37
# speedup vs baseline: 1.0044x; 1.0044x over previous
"""Trainium2 Bass kernel: attention with rotary embedding + XL memory (v3.3).

Model (B=2, T=1024, D=2048, H=16, hd=128, XL=1024):
  qkv = x @ w_qkv.T ; split q,k,v ; k_xl += pos_emb ; rope(q), rope(k)
  per head: scores = q @ [k_xl | k].T / sqrt(hd) ; softmax ; y = P @ [v_xl | v]
  out = y @ w_proj.T
sharding: 8 cores = 2 batches x 4 head-groups; host sums the 4 partial
output projections per batch.

v3 vs the 178.8us v2 (fp16-everywhere) kernel: the cost model charges
fp8e4/e5 DoubleRow matmuls 0.5 cycles per output row while packing TWO
128-deep k-tiles per instruction -- 4x fp16 MAC throughput. Straight fp8 is
numerically unusable (e4m3 ~2.7% RMS/element busts the 2e-2 gate), so the
big GEMMs use a residual-compensated decomposition:
    A@B ~= A8@B8 + Ar8@B8 + A8@Br8   (Ar8 = fp8(A - A8), cross term dropped)
Three DoubleRow instructions per two k-tiles = 0.75x fp16 cycles with
~1e-3 accuracy (device-validated). Applied to the qkv q/k projection, the
v projection, and the output projection (contractions 2048/2048/512). The
scores and AV matmuls keep fp16: their single 128-deep k-tile would make
DoubleRow pay parity-or-worse, and quantizing exp outputs on-chip is
engine-prohibitive.
  - all fp8 operand pairs except y are quantized on the HOST (x, w_qkv, wv,
    w_proj); main+residual are stacked along an `mr` axis of ONE dram
    tensor so each prefetch DMA covers both (fp8 pair == fp16 bytes).
  - scale management: weights are pre-scaled x64 on host so both fp8 tensors
    and their unscaled residuals sit in e4m3's normal range; the 1/64 is
    folded into the rope cos/sin tables (q/k), a DVE copy scale (v), and
    the host-side unshard divide (proj output is stored as 2048*out). The
    softmax denominator's `ones` reduction vector is 1/32 so the
    reciprocal broadcast yields 32/den and y16 = py*rbc = 32*y ~ unit RMS,
    putting y8's residual in fp8 range.
  - y is the only on-chip quantization: per (head, tb), y16 = py*rbc (DVE),
    y8 = cast (Pool mid-attention / ACT at the tail), yr8 = y16-y8 (DVE).

v3.3 schedule notes (all cost-model-trace driven; 178.8 -> 168.2us):
  - Pool/SWDGE descriptor generation is ~1.1us per dma_start and serial on
    the Pool engine, so prefetches are COALESCED (one DMA per wqk f-group
    covering main+resid, one for wv/wproj/vxl). w-f2 rides the fast
    SP/HWDGE queue head ahead of the x tb0 pieces; x tb1 + kxl sit BEHIND
    the weight stream on Pool so their transfers cannot steal bus from
    w f3..f7 (measured 4.9us PE stall when they issued early from SP).
  - phase-1 PE order tb0 f2..f7 -> tb0 f0/f1 (2-chain) -> tb1 f4/f5/f6
    (3-chain) -> tb1 f7,f0..f3: single chains consume only the weight
    stream (~200GB/s) while x tb0 lands; the x-hungry interleaved chains
    run once their tb's x is resident; k-groups of tb1 precede q-groups so
    attention tb0 (which reads k of both tbs) never waits on q-tb1 ropes.
  - all rope c16 PSUM->fp16 casts run on ACT: a DVE c16 at the phase tail
    holds PSUM slots hostage behind the DVE backlog and stalls the first
    attention scores (measured 2.7us).
  - per-head attention tail is fused (den+reciprocal -> deferred AVs ->
    broadcast -> normalize -> y8 cast -> yr8 sub) so head h's Pool/DVE/ACT
    chain drains under head h+1's AV matmuls; reserve=8 proj-tb0 fillers
    run after the chains to cover the ACT-backlog-gated tail before the
    proj-tb1 blocks need y8/yr8 (was a 4.3us PE stall at reserve=4).
  - ALL v-gemm groups fill attention-tb0 chunk slots (PE 2984ns vs ACT 4
    exps 2448ns per slot); proj-tb0 blocks fill attention tb1 starting at
    slot 3 (earlier ones would block the in-order PE queue on the tb0
    y-quant chain); out-DMAs alternate SP/Pool queues at the tail.
  Measured dead ends kept out: stream-interleaved SP byte order and a
  2-chain f2/f3 front (+12us), reserved fillers before the denominators
  (delays the AV->quant critical path, +4.6us), paired out-DMAs with an
  interleaved out layout (non-coalescable partition rows, +4us), outp pool
  below 8 bufs (DMA-completion rotation throttles proj blocks, +4us),
  denominators packed into one PSUM bank at partition offsets 0/32/64/96
  (walrus codegen rejects offset matmul outputs).
"""
import sys

sys.path.insert(0, "/opt/trn_rl_repo")

import numpy as np
import ml_dtypes

import concourse.bass as bass  # noqa: F401
import concourse.mybir as mybir
import concourse.tile as tile
from concourse import bacc
from concourse.bass import ts
from concourse.bass_utils import run_bass_kernel_spmd  # noqa: F401 (fallback)

F32 = mybir.dt.float32
F16 = mybir.dt.float16
F8 = mybir.dt.float8e4
AF = mybir.ActivationFunctionType
DR = mybir.MatmulPerfMode.DoubleRow
SUB = mybir.AluOpType.subtract
E4 = ml_dtypes.float8_e4m3

B, T, D = 2, 1024, 2048
H, HD, XL = 16, 128, 1024
HPC = 4                 # heads per core
CPB = 4                 # cores per batch
NCORES = 8
NCC = D // 128          # 16 contraction chunks (8 DoubleRow pairs)
NCJ = NCC // 2
SCALE = 1.0 / np.sqrt(HD)
WS = 64.0               # host weight prescale (folded back downstream)
YS = 32.0               # y prescale via ones=1/YS denominator reduction

_CACHE: dict = {}


def _build_nc():
    nc = bacc.Bacc("TRN2", target_bir_lowering=False, debug=False)

    x_d = nc.dram_tensor("x", [2, 128, 2, NCJ, 2, 512], F8,
                         kind="ExternalInput")
    wqk_d = nc.dram_tensor("wqk", [8, 128, 2, NCJ, 2, 128], F8,
                           kind="ExternalInput")
    wv_d = nc.dram_tensor("wv", [128, 2, NCJ, 2, 512], F8,
                          kind="ExternalInput")
    cs_d = nc.dram_tensor("cs", [2, 128, T], F16, kind="ExternalInput")
    kxl_d = nc.dram_tensor("kxl", [128, 4, XL], F16, kind="ExternalInput")
    vxl_d = nc.dram_tensor("vxl", [128, 8, 512], F16, kind="ExternalInput")
    wp_d = nc.dram_tensor("wp", [128, 2, 16, 2, 2, 128], F8,
                          kind="ExternalInput")
    out_d = nc.dram_tensor("out", [16, 2, 128, 512], F16, kind="ExternalOutput")

    gp = nc.gpsimd
    with tile.TileContext(nc) as tc, nc.allow_low_precision(
            reason="fp8 DoubleRow residual-compensated pipeline: ~2e-3 rel "
                   "err, gate is 2e-2"):
        with (
            tc.tile_pool(name="const", bufs=1) as const,
            tc.tile_pool(name="ropep", bufs=3) as ropep,
            tc.tile_pool(name="ptp", bufs=17) as ptp,
            tc.tile_pool(name="accp", bufs=8) as accp,
            tc.tile_pool(name="smallp", bufs=6) as smallp,
            tc.tile_pool(name="rbcp", bufs=4) as rbcp,
            tc.tile_pool(name="ynp", bufs=4) as ynp,
            tc.tile_pool(name="outp", bufs=8) as outp,
            tc.tile_pool(name="psum", bufs=4, space="PSUM") as psum,
            tc.tile_pool(name="pyp", bufs=4, space="PSUM") as pyp,
        ):
            # ---- persistent tiles (everything resident once loaded) ----
            cc = const.tile([128, T], F16, tag="cc")    # [cos; cos] / 64
            ss = const.tile([128, T], F16, tag="ss")    # [-sin; +sin] / 64
            ones = const.tile([128, 128], F16, tag="ones")  # = 1/YS
            qk = const.tile([128, 8, T], F16, tag="qk")   # roped qT 0-3, kT 4-7
            vsb = const.tile([128, 8, 512], F16, tag="vsb")  # v [t, d] natural
            y8sb = const.tile([128, 4, T], F8, tag="y8")     # 32*y fp8 main
            yr8sb = const.tile([128, 4, T], F8, tag="yr8")   # 32*y fp8 resid
            xB = const.tile([128, 2, 2, NCJ, 2, 512], F8, tag="x")
            wqkB = const.tile([128, 8, 2, NCJ, 2, 128], F8, tag="wqk")
            wvB = const.tile([128, 2, NCJ, 2, 512], F8, tag="wv")
            kxl = const.tile([128, 4, XL], F16, tag="kxl")
            vxl = const.tile([128, 8, 512], F16, tag="vxl")
            wpB = const.tile([128, 2, 16, 2, 2, 128], F8, tag="wp")

            gp.memset(ones[:], 1.0 / YS)

            # PE p-state warmup: dummy matmuls on `ones` while the first
            # DMAs land, so real matmuls start at the full 2.4GHz p-state
            # (the ramp needs ~3us of continuous PE busy). The dummy exp
            # pulls the ACT function-table load (1.3us) off the first real
            # exp at attention start.
            warm16 = ropep.tile([128, 512], F16, tag="c16", name="w16")
            for wu in range(2):
                pw = psum.tile([128, 512], F32, tag="ps", name="warm")
                for _ in range(13):
                    nc.tensor.matmul(pw[:, 0:128], ones[:], ones[:],
                                     start=True, stop=True)
                if wu == 0:
                    nc.scalar.activation(warm16[0:1, 0:128], pw[0:1, 0:128],
                                         AF.Exp, scale=SCALE)

            # ---- prefetch, priority order matched to the phase-1 PE order.
            # SP/HWDGE (565ns issue): x tb0 stream, then x tb1 + kxl.
            # Pool/SWDGE (~1.1us gen per DMA, serial on Pool): weights in
            # f-need order; each DMA covers a main+resid pair.
            # Stream-aligned prefetch: SP/HWDGE (565ns issue) carries the
            # phase-1-critical bytes in exact CONSUMPTION order (w f2, x
            # pieces interleaved with w f3..f7, then x tb1 + kxl), so the
            # front 2-chain is never byte-starved; Pool/SWDGE (~1.1us gen
            # per DMA, serial on Pool) trickles the rest.
            nc.sync.dma_start(wqkB[:, 2], wqk_d[2])
            for p2 in range(4):                      # x tb0, 2-j pieces
                nc.sync.dma_start(xB[:, 0, :, 2 * p2:2 * p2 + 2],
                                  x_d[0, :, :, 2 * p2:2 * p2 + 2])
            for f in (3, 4, 5, 6, 7):
                gp.dma_start(wqkB[:, f], wqk_d[f])
            gp.dma_start(cc[:], cs_d[0])
            gp.dma_start(ss[:], cs_d[1])
            gp.dma_start(wqkB[:, 0], wqk_d[0])
            gp.dma_start(wqkB[:, 1], wqk_d[1])
            gp.dma_start(xB[:, 1], x_d[1])           # x tb1 whole
            gp.dma_start(kxl[:], kxl_d[:])
            gp.dma_start(wvB[:], wv_d[:])
            gp.dma_start(vxl[:], vxl_d[:])
            gp.dma_start(wpB[:], wp_d[:])

            # ---- phase 1: q/k projection + rope ----
            def qk_mms_j(pmm, f, tb, j):
                # residual-compensated fp8 DoubleRow: one 256-deep k-tile
                # pair per instruction, 3 instructions per pair
                nc.tensor.matmul(pmm[:], wqkB[:, f, 0, j], xB[:, tb, 0, j],
                                 start=(j == 0), stop=False, perf_mode=DR)
                nc.tensor.matmul(pmm[:], wqkB[:, f, 1, j], xB[:, tb, 0, j],
                                 start=False, stop=False, perf_mode=DR)
                nc.tensor.matmul(pmm[:], wqkB[:, f, 0, j], xB[:, tb, 1, j],
                                 start=False, stop=(j == NCJ - 1),
                                 perf_mode=DR)

            def emit_rope(pmm, f, tb):
                # packed rope: new = P*[cos;cos] + swap(P)*[-sin;+sin].
                # cc/ss carry the 1/WS weight-prescale compensation, so
                # c16 holds 64*q and dst comes out at natural scale. c16
                # (ACT) casts PSUM->fp16 so the DVE combine runs in 4x
                # packed mode; the half-swap copies run on Pool.
                tbsl = ts(tb, 512)
                c16 = ropep.tile([128, 512], F16, tag="c16")
                nc.scalar.copy(c16[:], pmm[:])
                sw = ropep.tile([128, 512], F16, tag="sw")
                gp.tensor_copy(sw[0:64, :], c16[64:128, :])
                gp.tensor_copy(sw[64:128, :], c16[0:64, :])
                dst = qk[:, f, tbsl]
                t2 = ropep.tile([128, 512], F16, tag="t2")
                nc.vector.tensor_mul(dst, c16[:], cc[:, tbsl])
                nc.vector.tensor_mul(t2[:], sw[:], ss[:, tbsl])
                nc.vector.tensor_add(dst, dst, t2[:])

            def qk_group_chains(specs):
                # interleaved f-chains: PE consumes the incoming x/w byte
                # stream no faster than the 360GB/s transfer unit delivers
                pms = {}
                for fx, tbx in specs:
                    pms[(fx, tbx)] = psum.tile([128, 512], F32, tag="ps",
                                               name=f"pm{fx}{tbx}")
                for j in range(NCJ):
                    for fx, tbx in specs:
                        qk_mms_j(pms[(fx, tbx)], fx, tbx, j)
                for fx, tbx in specs:
                    emit_rope(pms[(fx, tbx)], fx, tbx)

            # tb0: f2/f3 as a 2-chain paced to the arriving x tb0 stream
            # (a single chain consumes x at 780GB/s vs the ~300GB/s bus),
            # f4..f7 single once x is resident, then f0/f1. tb1: f4/f5/f6
            # as a 3-chain paced to the x tb1 stream, k-groups (f4..f7)
            # before q-groups so attention tb0 (which needs k of both tbs)
            # never waits on q-tb1 ropes.
            for f in range(2, 8):
                qk_group_chains([(f, 0)])
            qk_group_chains([(0, 0), (1, 0)])
            qk_group_chains([(4, 1), (5, 1), (6, 1)])
            for f in (7, 0, 1, 2, 3):
                qk_group_chains([(f, 1)])

            # v in natural [t, d] layout. ALL v-gemm groups are deferred into
            # the attention-tb0 chunk slots (PE gap fillers). Half-width
            # (256 v-cols) gives 16 fillers for 16 slots; column half `hf`
            # covers heads 2hf..2hf+1. The PSUM->SBUF copy runs on DVE
            # (tensor_scalar 1/WS) because ACT's exp headroom in tb0 slots
            # is thin with the 0.75x fp8 fillers.
            def emit_v_group(tb, tt, hf):
                pv = psum.tile([128, 256], F32, tag="ps", name="pv")
                for j in range(NCJ):
                    nc.tensor.matmul(pv[:], xB[:, tb, 0, j, :, ts(tt, 128)],
                                     wvB[:, 0, j, :, ts(hf, 256)],
                                     start=(j == 0), stop=False, perf_mode=DR)
                    nc.tensor.matmul(pv[:], xB[:, tb, 1, j, :, ts(tt, 128)],
                                     wvB[:, 0, j, :, ts(hf, 256)],
                                     start=False, stop=False, perf_mode=DR)
                    nc.tensor.matmul(pv[:], xB[:, tb, 0, j, :, ts(tt, 128)],
                                     wvB[:, 1, j, :, ts(hf, 256)],
                                     start=False, stop=(j == NCJ - 1),
                                     perf_mode=DR)
                nc.vector.tensor_scalar_mul(vsb[:, tb * 4 + tt, ts(hf, 256)],
                                            pv[:], 1.0 / WS)

            v_fillers = [
                lambda tb=tb, tt=tt, hf=hf: emit_v_group(tb, tt, hf)
                for tb in range(2) for tt in range(4) for hf in range(2)]

            # ---- phase 2: attention + projection, interleaved ----
            def emit_proj(ob, tb, on_act=False, dma_gp=False):
                tbsl = ts(tb, 512)
                po = psum.tile([128, 512], F32, tag="ps")
                for yj in range(2):
                    y8p = y8sb[:, 2 * yj:2 * yj + 2, tbsl]
                    yr8p = yr8sb[:, 2 * yj:2 * yj + 2, tbsl]
                    nc.tensor.matmul(po[:], wpB[:, 0, ob, yj], y8p,
                                     start=(yj == 0), stop=False, perf_mode=DR)
                    nc.tensor.matmul(po[:], wpB[:, 1, ob, yj], y8p,
                                     start=False, stop=False, perf_mode=DR)
                    nc.tensor.matmul(po[:], wpB[:, 0, ob, yj], yr8p,
                                     start=False, stop=(yj == 1), perf_mode=DR)
                ot = outp.tile([128, 512], F16, tag="ot")
                # out is stored as WS*YS*out = 2048*out; host divides.
                # Pool/GPSIMD cannot read PSUM: fillers copy on DVE (ACT is
                # exp-saturated mid-attention); the tail copies on ACT.
                if on_act:
                    nc.scalar.copy(ot[:], po[:])
                else:
                    nc.vector.tensor_copy(ot[:], po[:])
                # the 16-block tail would queue 9us of serial SP DMA issue;
                # alternate the idle Pool/SWDGE queue to halve it
                if dma_gp:
                    gp.dma_start(out_d[ob, tb], ot[:])
                else:
                    nc.sync.dma_start(out_d[ob, tb], ot[:])

            def attn_quad(tb, fillers, every=2, reserve=0, act_quant=False):
                """Chunk-interleaved attention for all 4 heads; `fillers`
                are callables (or None placeholders) emitted inside chunk
                slots (PE gap fillers). The per-head tail (denominator,
                deferred AVs, reciprocal broadcast, normalize, y8/yr8 fp8
                quantization) is fused per head so head h's norm/quant chain
                (Pool/DVE/ACT) drains under head h+1..3's AV matmuls; the
                last `reserve` fillers run after it to cover the tail."""
                tbsl = ts(tb, 512)
                py, acc = {}, {}
                for h in range(4):
                    py[h] = pyp.tile([128, 512], F32, tag="py", name=f"py{h}")
                    acc[h] = accp.tile([128, 512], F16, tag="acc",
                                       name=f"acc{h}")
                fill = list(fillers)
                pend = {h: [] for h in range(4)}   # av deferred 3 chunks
                def emit_av(h):
                    pt_, lv_, kc_ = pend[h].pop(0)
                    nc.tensor.matmul(py[h][:], lv_, pt_[:],
                                     start=(kc_ == 0), stop=(kc_ == 15))
                for kc in range(16):
                    for h in range(4):
                        if kc < 8:
                            lk = kxl[:, h, ts(kc, 128)]
                            lv = vxl[:, kc, ts(h, 128)]
                        else:
                            lk = qk[:, 4 + h, ts(kc - 8, 128)]
                            lv = vsb[:, kc - 8, ts(h, 128)]
                        pss = psum.tile([128, 512], F32, tag="ps")
                        nc.tensor.matmul(pss[:], lk, qk[:, h, tbsl],
                                         start=True, stop=True)
                        pt = ptp.tile([128, 512], F16, tag="pt")
                        nc.scalar.activation(pt[:], pss[:], AF.Exp, scale=SCALE)
                        if kc == 0:
                            nc.vector.tensor_copy(acc[h][:], pt[:])
                        else:
                            nc.vector.tensor_add(acc[h][:], acc[h][:], pt[:])
                        if len(pend[h]) >= 3:
                            emit_av(h)
                        pend[h].append((pt, lv, kc))
                    if kc % every == every - 1 and len(fill) > reserve:
                        f = fill.pop(0)
                        if f is not None:
                            f()
                # denominators first (acc is final after kc15's add) so the
                # DVE reciprocals hide under the final AV matmuls. ones=1/YS
                # makes rec = YS/den so y16 = py*rbc = YS*y ~ unit RMS.
                recs = []
                for h in range(4):
                    pden_t = psum.tile([128, 512], F32, tag="ps")
                    nc.tensor.matmul(pden_t[0:1, :], ones[:, 0:1], acc[h][:],
                                     start=True, stop=True)
                    rec = smallp.tile([1, 512], F16, tag="rec")
                    nc.vector.reciprocal(rec[:], pden_t[0:1, :])
                    recs.append(rec)
                # per-head tail: deferred AVs -> reciprocal broadcast on Pool
                # -> normalize (frees the py bank) -> y8 cast (Pool mid-
                # attention, ACT at the tail) -> yr8 residual on DVE. Head
                # h's non-PE chain hides under head h+1's AV matmuls.
                for h in range(4):
                    while pend[h]:
                        emit_av(h)
                    rbc = rbcp.tile([128, 512], F16, tag="rbc")
                    gp.partition_broadcast(rbc[:], recs[h][:])
                    y16 = ynp.tile([128, 512], F16, tag="y16")
                    nc.vector.tensor_mul(y16[:], py[h][:], rbc[:])
                    dst8 = y8sb[:, h, tbsl]
                    if act_quant:
                        nc.scalar.copy(dst8, y16[:])
                    else:
                        gp.tensor_copy(dst8, y16[:])
                    nc.vector.tensor_tensor(yr8sb[:, h, tbsl], y16[:],
                                            dst8, SUB)
                # reserved fillers: independent PE work emitted after the
                # per-head chains to cover their Pool/DVE/ACT latency
                while fill:
                    f = fill.pop(0)
                    if f is not None:
                        f()

            attn_quad(0, v_fillers, every=1)
            # proj-tb0 fillers skip the first 3 tb1 slots (tb0's y8/yr8
            # quant chain is still draining; an early proj matmul would
            # block the in-order PE queue on it)
            # reserve=8: the tb1 y-quant chain is gated by the ACT exp
            # backlog (~5us); 8 reserved proj-tb0 blocks of PE work cover it
            attn_quad(1, [None] * 3
                      + [lambda ob=ob: emit_proj(ob, 0) for ob in range(16)],
                      every=1, reserve=8, act_quant=True)
            for ob in range(16):                  # proj tb1 (tail)
                emit_proj(ob, 1, on_act=True, dma_gp=(ob % 2 == 0))

    nc.compile()
    return nc


def _get_nc():
    if "nc" not in _CACHE:
        _CACHE["nc"] = _build_nc()
    return _CACHE["nc"]


_PERM = np.concatenate([np.arange(0, HD, 2), np.arange(1, HD, 2)])
_PP = np.concatenate([_PERM + i * HD for i in range(HPC)])  # per-head-block


def _q8mr(a, axis):
    """fp8 e4m3 main + unscaled residual, stacked along a new `mr` axis."""
    a8 = a.astype(E4)
    ar8 = (a - a8.astype(np.float32)).astype(E4)
    return np.ascontiguousarray(np.stack([a8, ar8], axis=axis))


def make_in_maps(x, cos, sin, k_xl, v_xl, pos_emb, w_qkv, w_proj):
    """Host-side shard + layout prep + fp8 quantization: one input dict per
    core."""
    x = np.asarray(x, np.float32)
    cos = np.asarray(cos, np.float32)
    sin = np.asarray(sin, np.float32)
    k_xl = np.asarray(k_xl, np.float32) + np.asarray(pos_emb, np.float32)
    v_xl = np.asarray(v_xl, np.float32)
    w_qkv = np.asarray(w_qkv, np.float32)
    w_proj = np.asarray(w_proj, np.float32)

    # cs[0] = [cos; cos]/WS ; cs[1] = [-sin; +sin]/WS  (packed-rope factors
    # with the w_qkv x64 prescale folded back)
    cs = np.ascontiguousarray(np.stack([
        np.concatenate([cos.T, cos.T], axis=0),
        np.concatenate([-sin.T, sin.T], axis=0),
    ]) / WS).astype(np.float16)

    in_maps = []
    for c in range(NCORES):
        b, g = c // CPB, c % CPB
        h0 = g * HPC
        cols = slice(h0 * HD, (h0 + HPC) * HD)

        # x: [tb, pi, mr, cj, kt, tl] fp8 pair
        x_arr = np.ascontiguousarray(
            x[b].T.reshape(NCJ, 2, 128, 2, 512).transpose(3, 2, 0, 1, 4))
        x_q = _q8mr(x_arr, 2)
        # w_q/w_k rows for this head group, rope-permuted, x WS;
        # [f, pi, mr, cj, kt, fcol] fp8 pair
        wq = w_qkv[0 * D + h0 * HD:0 * D + (h0 + HPC) * HD][_PP]
        wk = w_qkv[1 * D + h0 * HD:1 * D + (h0 + HPC) * HD][_PP]
        wqk_rows = np.concatenate([wq, wk], axis=0) * WS  # [1024, D]
        wqk_arr = np.ascontiguousarray(
            wqk_rows.reshape(8, 128, NCJ, 2, 128).transpose(0, 4, 2, 3, 1))
        wqk_q = _q8mr(wqk_arr, 2)
        # w_v rows (unpermuted) x WS; [pi, mr, cj, kt, vcol] fp8 pair
        wv_rows = w_qkv[2 * D + h0 * HD:2 * D + (h0 + HPC) * HD] * WS
        wv_arr = np.ascontiguousarray(
            wv_rows.T.reshape(NCJ, 2, 128, 512).transpose(2, 0, 1, 3))
        wv_q = _q8mr(wv_arr, 1)
        # k_xl (pos already added): permuted cols, transposed; [pi, j, t]
        kxlT = k_xl[b][:, cols][:, _PP].T  # [512, XL]
        kxl_arr = np.ascontiguousarray(
            kxlT.reshape(4, 128, XL).transpose(1, 0, 2)).astype(np.float16)
        # v_xl natural; [pi, j, col]
        vxl_arr = np.ascontiguousarray(
            v_xl[b][:, cols].reshape(8, 128, 512).transpose(1, 0, 2)
        ).astype(np.float16)
        # w_proj column block, transposed, x WS; [pi, mr, ob, ycj, kt, ocol]
        wprojT = w_proj[:, cols].T * WS  # [512, D]
        wp_arr = np.ascontiguousarray(
            wprojT.reshape(2, 2, 128, 16, 128).transpose(2, 3, 0, 1, 4))
        wp_q = _q8mr(wp_arr, 1)

        in_maps.append({
            "x": x_q, "wqk": wqk_q, "wv": wv_q, "cs": cs,
            "kxl": kxl_arr, "vxl": vxl_arr, "wp": wp_q,
        })
    return in_maps


def unshard(results):
    """results: list of 8 dicts with 'out' [16, 2, 128, 512] -> [B, T, D].
    Device partials are scaled by WS*YS; divide once after summing."""
    out = np.zeros((B, T, D), np.float32)
    for c in range(NCORES):
        b = c // CPB
        outT = np.asarray(results[c]["out"]).transpose(0, 2, 1, 3).reshape(D, T)
        out[b] += outT.T
    out /= (WS * YS)
    return out


def _get_runner():
    """Persistent jitted 8-core executable (avoids per-call retrace of the
    bass2jax lowering; the NEFF itself is cached by neuronx-cc)."""
    if "runner" in _CACHE:
        return _CACHE["runner"]
    import jax
    import jax.numpy as jnp
    from jax.sharding import Mesh, PartitionSpec, NamedSharding
    from jax.experimental.shard_map import shard_map
    from concourse.bass2jax import (_bass_exec_p, partition_id_tensor,
                                    install_neuronx_cc_hook)

    nc = _get_nc()
    install_neuronx_cc_hook()
    in_names, out_names, out_avals, zero_shapes = [], [], [], []
    for alloc in nc.m.functions[0].allocations:
        if not isinstance(alloc, mybir.MemoryLocationSet):
            continue
        name = alloc.memorylocations[0].name
        if alloc.kind == "ExternalInput":
            if nc.partition_id_tensor is None or \
                    name != nc.partition_id_tensor.name:
                in_names.append(name)
        elif alloc.kind == "ExternalOutput":
            shape = tuple(alloc.tensor_shape)
            np_dt = mybir.dt.np(alloc.dtype)
            out_names.append(name)
            out_avals.append(jax.core.ShapedArray(shape, np_dt))
            zero_shapes.append((shape, np_dt))
    n_params, n_outs = len(in_names), len(out_names)
    all_in = in_names + out_names
    if nc.partition_id_tensor is not None:
        all_in = all_in + [nc.partition_id_tensor.name]

    def _body(*args):
        operands = list(args)
        if nc.partition_id_tensor is not None:
            operands.append(partition_id_tensor())
        return tuple(_bass_exec_p.bind(
            *operands, out_avals=tuple(out_avals), in_names=tuple(all_in),
            out_names=tuple(out_names), lowering_input_output_aliases=(),
            sim_require_finite=True, sim_require_nnan=True, nc=nc))

    devices = jax.devices()[:NCORES]
    mesh = Mesh(np.asarray(devices), ("core",))
    fn = jax.jit(
        shard_map(_body, mesh=mesh,
                  in_specs=(PartitionSpec("core"),) * (n_params + n_outs),
                  out_specs=(PartitionSpec("core"),) * n_outs,
                  check_rep=False),
        donate_argnums=tuple(range(n_params, n_params + n_outs)),
        keep_unused=True)
    sharding = NamedSharding(mesh, PartitionSpec("core"))
    zfn = jax.jit(
        lambda: tuple(jnp.zeros((NCORES * s[0], *s[1:]), d)
                      for s, d in zero_shapes),
        out_shardings=(sharding,) * n_outs)
    runner = (fn, zfn, in_names, out_names, out_avals, sharding)
    _CACHE["runner"] = runner
    return runner


def kernel(x, cos, sin, k_xl, v_xl, pos_emb, w_qkv, w_proj, is_causal=0,
           **_ignored):
    # is_causal is 0 for this problem spec (fill=arange, shape []); the
    # non-causal path is the only one implemented.
    import jax
    in_maps = make_in_maps(x, cos, sin, k_xl, v_xl, pos_emb, w_qkv, w_proj)
    fn, zfn, in_names, out_names, out_avals, sharding = _get_runner()
    concat_in = [
        jax.device_put(
            np.concatenate([in_maps[c][nm] for c in range(NCORES)], axis=0),
            sharding)
        for nm in in_names]
    outs = fn(*concat_in, *zfn())
    results = [
        {nm: np.asarray(outs[i]).reshape(NCORES, *out_avals[i].shape)[c]
         for i, nm in enumerate(out_names)}
        for c in range(NCORES)]
    _CACHE["last_results"] = None
    return unshard(results)
